# revision 8
# baseline (speedup 1.0000x reference)
"""AssociativeEmbeddingLoss on 8 TRN2 NeuronCores (Bass/Tile kernel) — v3.

Entry point: kernel(**inputs) -> np.ndarray (3,) = (pull, push, scale),
matching the reference. Data-parallel on batch dim N=16 -> 2 images per
core; per-image partials are averaged on the host.

Design (vs the 39.4us 9-call baseline):
  - Only VISIBLE joints are gathered: invisible ones are multiplied by
    zero downstream anyway, so the host compacts the ~1020 (person,
    joint) pairs to the ~510 visible ones. 640 descriptor capacity
    (8 sigma above the Binomial(1020,1/2) mean) -> FIVE indirect-DMA
    calls instead of nine; a >640 overflow (never for random inputs)
    falls back to a lazily-built 9-call variant.
  - The visibility mask is folded into per-block 0/1 selector matrices:
    as each 128-descriptor gather block lands, one PE matmul
    accumulates both U = sum(vis*g) and V = sum(vis*g^2) into PSUM
    ([64,32], rhs = [G_c | G_c^2] via a strided view), entirely hidden
    under the remaining gather stream. No 128->60 half-merge needed.
  - Everything derivable from visibility counts / box_scales alone
    (reciprocals, valid masks, normalized scale targets tgt, per-image
    1/n factors) is precomputed on the host into one constant-block
    DMA. The device only computes what needs gathered tag values.
  - All activations used (Exp/Abs) live in one activation-table set, so
    one hidden table load replaces the baseline's five (two of which
    sat on the critical path).
  - rsqrt(||U||^2) for the scale loss is a DVE Newton iteration seeded
    by the int32 bit trick - no Sqrt table set, no Act round-trip.
  - The push-loss pair mask folds into the Gram matmul via +-64.0
    image-indicator feature columns (64^2=4096 exactly cancels the 2048
    h-constants in fp32); the diagonal exp(0)=1 surplus per valid
    person is cancelled by host-constant pseudo-rows in the final
    per-image reduction matmul.
"""

import numpy as np

import concourse.bacc as bacc
import concourse.mybir as mybir
import concourse.tile as tile
from concourse.bass import IndirectOffsetOnAxis
from concourse.bass_utils import run_bass_kernel_spmd

F32 = mybir.dt.float32
I32 = mybir.dt.int32
AF = mybir.ActivationFunctionType
ALU = mybir.AluOpType

S = 16      # scale-embedding dim
K = 17      # joints
M = 30      # persons per image
N = 16      # batch
L = 69632   # flattened tag locations per image
N_CORES = 8
N_IMG = N // N_CORES    # images per core
JR = 64                 # person rows per core (2 images x 32, rows 30/31 dead)
CB = 64.0               # c; c^2 = 4096 exact

# feature flags (validated by HW probes; flip off to use safe fallbacks)
USE_NEWTON = True       # DVE bit-trick rsqrt instead of Sqrt activation table
USE_TTR = False         # fused tensor_tensor_reduce: custom-DVE op, crashes this runtime
USE_STT = True          # fused scalar_tensor_tensor
USE_HDUAL = True        # single strided dual-column write of h into Wa and Wb
USE_F32R = False        # verifier requires producer-side fp32r rounding

# TLC (constant-block) column layout, [66, TLC_W]
C_IDENT = 0      # 0:64 identity for the PE transpose
C_IND = 64       # 64:66 image one-hot; rows 64/65 = eye(2) pseudo
C_WA = 68        # 68:88  Wa: [-mean(16) | h | 1 | c*ind(2)]
C_WB = 100       # 100:120 Wb: [ mean(16) | 1 | h | -c*ind(2)]
C_TGT = 132      # 132:148 normalized scale target
C_RECIP = 148
C_NR = 149
C_RRVP = 150
C_H0 = 151
C_HV2 = 152
C_VRN = 153
C_CPUSH = 154
C_NVRN = 155
C_STAT = 156     # 156:159 stat cols: pull | push | scale; rows 64/65 pseudo
TLC_W = 160


def build_nc(n_blk=5):
    """n_blk gather blocks of 128 descriptors each."""
    nc = bacc.Bacc("TRN2", target_bir_lowering=False, debug=False)

    tags = nc.dram_tensor("tags", [N_IMG * L, S], F32, kind="ExternalInput")
    j2d = nc.dram_tensor("j2", [128, n_blk], I32, kind="ExternalInput")
    seld = nc.dram_tensor("sel", [128, n_blk * JR], F32, kind="ExternalInput")
    tlcd = nc.dram_tensor("tlc", [JR + 2, TLC_W], F32, kind="ExternalInput")
    out = nc.dram_tensor("out", [N_IMG, 3], F32, kind="ExternalOutput")

    with tile.TileContext(nc) as tc:
        with (
            tc.tile_pool(name="sb", bufs=1) as sb,
            tc.tile_pool(name="ps", bufs=1, space="PSUM") as ps,
        ):
            # ---- loads. j2 gates the gather stream -> first on Act queue;
            # sel on the idle sync queue; one hidden ACT table load (Exp
            # set, which also contains Abs) via the warmup activation ----
            j2 = sb.tile([128, n_blk], I32, tag="j2")
            nc.scalar.dma_start(j2[:], j2d.ap())
            tlc = sb.tile([JR + 2, TLC_W], F32, tag="tlc")
            nc.scalar.dma_start(tlc[:], tlcd.ap())
            sel = sb.tile([128, n_blk * JR], F32, tag="sel")
            nc.sync.dma_start(sel[:], seld.ap())

            warm = sb.tile([1, 2], F32, tag="warm")
            nc.vector.memset(warm[:, 0:1], 1.0)
            nc.scalar.activation(out=warm[:, 1:2], in_=warm[:, 0:1], func=AF.Exp)

            # ---- gather stream + per-block accumulation ----
            # Per block: mmU accumulates U = sel_c^T @ G_c straight off the
            # landed data (no DVE hop on the critical path); mmV accumulates
            # V = sel_c^T @ G_c^2 from the DVE square.
            F32M = mybir.dt.float32r if USE_F32R else F32
            GS = sb.tile([128, 2 * S * n_blk], F32, tag="GS")
            uvU = ps.tile([JR, S], F32, tag="uvU")
            uvV = ps.tile([JR, S], F32, tag="uvV")
            for c in range(n_blk):
                nc.gpsimd.indirect_dma_start(
                    out=GS[:, c * S : (c + 1) * S],
                    out_offset=None,
                    in_=tags.ap(),
                    in_offset=IndirectOffsetOnAxis(ap=j2[:, c : c + 1], axis=0),
                )
            sqo = n_blk * S
            for c in range(n_blk):
                g_c = GS[:, c * S : (c + 1) * S]
                sq_c = GS[:, sqo + c * S : sqo + (c + 1) * S]
                nc.vector.tensor_mul(out=sq_c, in0=g_c, in1=g_c)
                nc.tensor.matmul(
                    out=uvU[:],
                    lhsT=sel[:, c * JR : (c + 1) * JR].bitcast(F32M),
                    rhs=g_c.bitcast(F32M),
                    start=(c == 0),
                    stop=(c == n_blk - 1),
                )
                nc.tensor.matmul(
                    out=uvV[:],
                    lhsT=sel[:, c * JR : (c + 1) * JR].bitcast(F32M),
                    rhs=sq_c.bitcast(F32M),
                    start=(c == 0),
                    stop=(c == n_blk - 1),
                )

            # ---- per-person stats (U/V read straight from PSUM) ----
            tj = tlc[0:JR, :]
            # push feature data columns (consts arrived via the TLC DMA)
            nc.vector.tensor_scalar_mul(
                out=tj[:, C_WB : C_WB + S], in0=uvU[:],
                scalar1=tj[:, C_RECIP : C_RECIP + 1],
            )
            nc.vector.tensor_scalar_mul(
                out=tj[:, C_WA : C_WA + S], in0=uvU[:],
                scalar1=tj[:, C_NR : C_NR + 1],
            )
            q = sb.tile([JR, 1], F32, tag="q")
            scrq = sb.tile([JR, S], F32, tag="scrq")
            nc.scalar.activation(
                out=scrq[:], in_=uvU[:], func=AF.Square, accum_out=q[:]
            )
            absU = sb.tile([JR, S], F32, tag="absU")
            nc.scalar.activation(out=absU[:], in_=uvU[:], func=AF.Abs)
            sg = sb.tile([JR, 1], F32, tag="sg")
            nc.vector.reduce_sum(out=sg[:], in_=uvV[:], axis=mybir.AxisListType.X)
            scr16 = sb.tile([JR, S], F32, tag="scr16")

            if USE_HDUAL:
                hview = tj[:, C_WA + S : C_WA + S + 2 * 33].rearrange(
                    "p (a b) -> p a b", a=2
                )[:, :, 0:1]
                qb = q[:].rearrange("p (a b) -> p a b", a=1).to_broadcast(
                    [JR, 2, 1]
                )
                nc.vector.tensor_scalar(
                    out=hview, in0=qb,
                    scalar1=tj[:, C_H0 : C_H0 + 1], scalar2=tj[:, C_HV2 : C_HV2 + 1],
                    op0=ALU.mult, op1=ALU.add,
                )
            else:
                nc.vector.tensor_scalar(
                    out=tj[:, C_WA + S : C_WA + S + 1], in0=q[:],
                    scalar1=tj[:, C_H0 : C_H0 + 1], scalar2=tj[:, C_HV2 : C_HV2 + 1],
                    op0=ALU.mult, op1=ALU.add,
                )
                nc.scalar.copy(
                    out=tj[:, C_WB + S + 1 : C_WB + S + 2],
                    in_=tj[:, C_WA + S : C_WA + S + 1],
                )

            # ---- push: one transpose, two aligned copies, Gram, Exp ----
            tp = ps.tile([64, JR], F32, tag="tp")
            nc.tensor.transpose(
                out=tp[:], in_=tj[:, C_WA : C_WA + 64], identity=tj[:, 0:JR]
            )
            X = sb.tile([20, JR], F32, tag="X")
            nc.scalar.copy(out=X[:], in_=tp[0:20, :])
            Y = sb.tile([20, JR], F32, tag="Y")
            nc.scalar.copy(out=Y[:], in_=tp[32:52, :])
            F32M2 = mybir.dt.float32r if USE_F32R else F32
            dh = ps.tile([JR, JR], F32, tag="dh")
            nc.tensor.matmul(
                out=dh[:], lhsT=X[:].bitcast(F32M2), rhs=Y[:].bitcast(F32M2),
                start=True, stop=True,
            )

            # ---- scale branch: rsqrt(q) ----
            rq = sb.tile([JR, 1], F32, tag="rq")
            mq = sb.tile([JR, 1], F32, tag="mq")
            nc.vector.tensor_scalar_max(out=mq[:], in0=q[:], scalar1=1e-30)
            if USE_NEWTON:
                ti = sb.tile([JR, 1], I32, tag="ti")
                nc.vector.tensor_single_scalar(
                    out=ti[:], in_=mq[:].bitcast(I32), scalar=1,
                    op=ALU.logical_shift_right,
                )
                yi = sb.tile([JR, 1], I32, tag="yi")
                nc.vector.tensor_scalar(
                    out=yi[:], in0=ti[:], scalar1=-1, scalar2=0x5F3759DF,
                    op0=ALU.mult, op1=ALU.add,
                )
                y0 = yi[:].bitcast(F32)
                y2 = sb.tile([JR, 1], F32, tag="y2")
                e = sb.tile([JR, 1], F32, tag="e")
                f = sb.tile([JR, 1], F32, tag="f")
                nc.vector.tensor_mul(out=y2[:], in0=y0, in1=y0)
                nc.vector.tensor_mul(out=e[:], in0=mq[:], in1=y2[:])
                nc.vector.tensor_scalar(
                    out=f[:], in0=e[:], scalar1=-0.5, scalar2=1.5,
                    op0=ALU.mult, op1=ALU.add,
                )
                # one Newton step: rel err ~2e-3 (seed ~3.4%), inside budget
                nc.vector.tensor_mul(out=rq[:], in0=y0, in1=f[:])
            else:
                sq_ = sb.tile([JR, 1], F32, tag="sq_")
                nc.scalar.sqrt(out=sq_[:], in_=mq[:])
                nc.vector.reciprocal(out=rq[:], in_=sq_[:])

            A = sb.tile([JR, 1], F32, tag="A")
            if USE_TTR:
                nc.vector.tensor_tensor_reduce(
                    out=scr16[:], in0=absU[:], in1=tj[:, C_TGT : C_TGT + S],
                    scale=1.0, scalar=0.0, op0=ALU.mult, op1=ALU.add,
                    accum_out=A[:],
                )
            else:
                nc.vector.tensor_mul(
                    out=scr16[:], in0=absU[:], in1=tj[:, C_TGT : C_TGT + S]
                )
                nc.vector.reduce_sum(
                    out=A[:], in_=scr16[:], axis=mybir.AxisListType.X
                )
            d12 = sb.tile([JR, 1], F32, tag="d12")
            nc.vector.tensor_mul(out=d12[:], in0=A[:], in1=rq[:])
            # scale stat = vrn - d12*vrn
            if USE_STT:
                nc.vector.scalar_tensor_tensor(
                    out=tj[:, C_STAT + 2 : C_STAT + 3], in0=d12[:],
                    scalar=tj[:, C_NVRN : C_NVRN + 1],
                    in1=tj[:, C_VRN : C_VRN + 1],
                    op0=ALU.mult, op1=ALU.add,
                )
            else:
                t2 = sb.tile([JR, 1], F32, tag="t2")
                nc.vector.tensor_scalar(
                    out=t2[:], in0=d12[:], scalar1=tj[:, C_NVRN : C_NVRN + 1],
                    scalar2=None, op0=ALU.mult,
                )
                nc.vector.tensor_scalar(
                    out=tj[:, C_STAT + 2 : C_STAT + 3], in0=t2[:],
                    scalar1=tj[:, C_VRN : C_VRN + 1], scalar2=None, op0=ALU.add,
                )
            # pull stat = (q*nr + sg) * rrvp
            p1 = sb.tile([JR, 1], F32, tag="p1")
            if USE_STT:
                nc.vector.scalar_tensor_tensor(
                    out=p1[:], in0=q[:], scalar=tj[:, C_NR : C_NR + 1],
                    in1=sg[:], op0=ALU.mult, op1=ALU.add,
                )
            else:
                t3 = sb.tile([JR, 1], F32, tag="t3")
                nc.vector.tensor_scalar(
                    out=t3[:], in0=q[:], scalar1=tj[:, C_NR : C_NR + 1],
                    scalar2=None, op0=ALU.mult,
                )
                nc.vector.tensor_add(out=p1[:], in0=t3[:], in1=sg[:])
            nc.vector.tensor_scalar_mul(
                out=tj[:, C_STAT : C_STAT + 1], in0=p1[:],
                scalar1=tj[:, C_RRVP : C_RRVP + 1],
            )

            # push stat: row-sums of exp(-2*Gram + ln(cpush)) accumulate
            # straight into the stat column (bias folds the cpush scale)
            eo = sb.tile([JR, JR], F32, tag="eo")
            nc.scalar.activation(
                out=eo[:], in_=dh[:], func=AF.Exp, scale=-2.0,
                bias=tj[:, C_CPUSH : C_CPUSH + 1],
                accum_out=tj[:, C_STAT + 1 : C_STAT + 2],
            )

            # ---- per-image reduction (pseudo-rows fold the diagonal fix) ----
            fsp = ps.tile([N_IMG, 3], F32, tag="fsp")
            nc.tensor.matmul(
                out=fsp[:], lhsT=tlc[:, C_IND : C_IND + 2],
                rhs=tlc[:, C_STAT : C_STAT + 3], start=True, stop=True,
            )
            ob = sb.tile([N_IMG, 3], F32, tag="ob")
            nc.vector.tensor_copy(out=ob[:], in_=fsp[:])
            nc.sync.dma_start(out.ap(), ob[:])

    nc.compile()
    return nc


def _prep_core(tags_c, joints_c, box_c, sd, n_blk):
    """Host-side shard prep: compacted visible-joint gather list, per-block
    selectors, and the constant block. Returns None if the visible count
    exceeds this build's capacity (caller rebuilds with more blocks)."""
    cap = 128 * n_blk
    tags2 = np.ascontiguousarray(
        np.asarray(tags_c, dtype=np.float32).reshape(N_IMG * L, S)
    )
    jl = np.asarray(joints_c[..., 0], dtype=np.int64)      # [2, 30, 17]
    vis = np.asarray(joints_c[..., 1]) > 0
    loc = (jl + (np.arange(N_IMG) * L)[:, None, None]).astype(np.int64)

    img_r, m_r, k_r = np.nonzero(vis)          # visible (img, person, joint)
    V = img_r.shape[0]
    if V > cap:
        return None
    jrow = img_r * 32 + m_r                    # person row 0..63
    locv = loc[img_r, m_r, k_r].astype(np.int32)

    j2 = np.zeros((128, n_blk), np.int32)
    selm = np.zeros((128, n_blk * JR), np.float32)
    fi = np.arange(V)
    p_i, c_i = fi % 128, fi // 128
    j2[p_i, c_i] = locv
    selm[p_i, c_i * JR + jrow] = 1.0

    visf = vis.reshape(N_IMG * M, K).astype(np.float32)
    cnt_pm = visf.sum(1).reshape(N_IMG, M)     # [2, 30]
    cnt = np.zeros((N_IMG, 32), np.float32)
    cnt[:, 0:M] = cnt_pm
    cnt = cnt.reshape(JR)
    recip = (1.0 / np.maximum(cnt, 1.0)).astype(np.float32)
    valid = (cnt > 0).astype(np.float32)
    imgr = np.arange(JR) // 32
    nv = np.array([valid[imgr == i].sum() for i in range(N_IMG)], np.float32)
    rn = (1.0 / np.maximum(nv, 1.0)).astype(np.float32)
    rp = (1.0 / np.maximum(nv * (nv - 1.0), 1.0)).astype(np.float32)
    ge2 = (nv >= 2.0).astype(np.float32)
    cpush = 0.5 * rp * ge2

    box = np.zeros((N_IMG, 32), np.float32)
    box[:, 0:M] = np.asarray(box_c, dtype=np.float32).reshape(N_IMG, M)
    box = box.reshape(JR)
    sd = np.asarray(sd, dtype=np.float32).reshape(S)
    gap = np.abs(box[:, None] - sd[None, :]).astype(np.float32)
    r = (np.float32(1.0) / (gap + np.float32(1e-10))).astype(np.float32)
    nrm = np.sqrt((r * r).sum(1, dtype=np.float32))
    tgt = r / np.maximum(nrm, np.float32(1e-12))[:, None]

    tlc = np.zeros((JR + 2, TLC_W), np.float32)
    pj = np.arange(JR)
    tlc[0:JR, 0:JR] = np.eye(JR, dtype=np.float32)
    tlc[pj, C_IND + imgr] = 1.0
    tlc[JR, C_IND] = 1.0
    tlc[JR + 1, C_IND + 1] = 1.0
    tlc[0:JR, C_WA + S + 1] = 1.0
    tlc[pj, C_WA + S + 2 + imgr] = CB
    tlc[0:JR, C_WB + S] = 1.0
    tlc[pj, C_WB + S + 2 + imgr] = -CB
    tlc[0:JR, C_TGT : C_TGT + S] = tgt
    tlc[0:JR, C_RECIP] = recip
    tlc[0:JR, C_NR] = -recip
    tlc[0:JR, C_RRVP] = (recip / S) * valid * rn[imgr]
    tlc[0:JR, C_H0] = 0.5 * recip * recip
    tlc[0:JR, C_HV2] = 4096.0 * (1.0 - valid) + 2048.0
    tlc[0:JR, C_VRN] = valid * rn[imgr]
    lncp = np.where(cpush > 0, np.log(np.maximum(cpush, 1e-38)), -1e30).astype(
        np.float32
    )
    tlc[0:JR, C_CPUSH] = lncp[imgr]
    tlc[0:JR, C_NVRN] = -tlc[0:JR, C_VRN]
    tlc[JR, C_STAT + 1] = -cpush[0] * nv[0]
    tlc[JR + 1, C_STAT + 1] = -cpush[1] * nv[1]
    return {"tags": tags2, "j2": j2, "sel": selm, "tlc": tlc}


_NC_CACHE = {}


def _get_nc(n_blk):
    if n_blk not in _NC_CACHE:
        _NC_CACHE[n_blk] = build_nc(n_blk)
    return _NC_CACHE[n_blk]


def kernel(tags, joints, box_scales, scale_dist, _trace=False):
    """Full-input entry point; shards across 8 NeuronCores and gathers."""
    tags = np.asarray(tags)
    joints = np.asarray(joints)
    box_scales = np.asarray(box_scales)
    scale_dist = np.asarray(scale_dist)

    for n_blk in (5, 9):  # 9-block fallback only if >640 joints are visible
        in_maps = [
            _prep_core(
                tags[N_IMG * c : N_IMG * (c + 1)],
                joints[N_IMG * c : N_IMG * (c + 1)],
                box_scales[N_IMG * c : N_IMG * (c + 1)],
                scale_dist,
                n_blk,
            )
            for c in range(N_CORES)
        ]
        if all(m is not None for m in in_maps):
            break

    res = run_bass_kernel_spmd(
        _get_nc(n_blk), in_maps, core_ids=list(range(N_CORES)), trace=_trace
    )
    parts = np.concatenate(
        [res.results[c]["out"] for c in range(N_CORES)], axis=0
    )  # [N, 3]
    final = parts.mean(axis=0).astype(np.float32)
    if _trace:
        return final, res
    return final


# revision 10
# speedup vs baseline: 1.0611x; 1.0611x over previous
"""AssociativeEmbeddingLoss on 8 TRN2 NeuronCores (Bass/Tile kernel) — v3.

Entry point: kernel(**inputs) -> np.ndarray (3,) = (pull, push, scale),
matching the reference. Data-parallel on batch dim N=16 -> 2 images per
core; per-image partials are averaged on the host.

Design (vs the 39.4us 9-call baseline):
  - Only VISIBLE joints are gathered: invisible ones are multiplied by
    zero downstream anyway, so the host compacts the ~1020 (person,
    joint) pairs to the ~510 visible ones. 640 descriptor capacity
    (8 sigma above the Binomial(1020,1/2) mean) -> FIVE indirect-DMA
    calls instead of nine; a >640 overflow (never for random inputs)
    falls back to a lazily-built 9-call variant.
  - The visibility mask is folded into per-block 0/1 selector matrices:
    as each 128-descriptor gather block lands, one PE matmul
    accumulates both U = sum(vis*g) and V = sum(vis*g^2) into PSUM
    ([64,32], rhs = [G_c | G_c^2] via a strided view), entirely hidden
    under the remaining gather stream. No 128->60 half-merge needed.
  - Everything derivable from visibility counts / box_scales alone
    (reciprocals, valid masks, normalized scale targets tgt, per-image
    1/n factors) is precomputed on the host into one constant-block
    DMA. The device only computes what needs gathered tag values.
  - All activations used (Exp/Abs) live in one activation-table set, so
    one hidden table load replaces the baseline's five (two of which
    sat on the critical path).
  - rsqrt(||U||^2) for the scale loss is a DVE Newton iteration seeded
    by the int32 bit trick - no Sqrt table set, no Act round-trip.
  - The push-loss pair mask folds into the Gram matmul via +-64.0
    image-indicator feature columns (64^2=4096 exactly cancels the 2048
    h-constants in fp32); the diagonal exp(0)=1 surplus per valid
    person is cancelled by host-constant pseudo-rows in the final
    per-image reduction matmul.
"""

import numpy as np

import concourse.bacc as bacc
import concourse.mybir as mybir
import concourse.tile as tile
from concourse.bass import IndirectOffsetOnAxis
from concourse.bass_utils import run_bass_kernel_spmd

F32 = mybir.dt.float32
I32 = mybir.dt.int32
AF = mybir.ActivationFunctionType
ALU = mybir.AluOpType

S = 16      # scale-embedding dim
K = 17      # joints
M = 30      # persons per image
N = 16      # batch
L = 69632   # flattened tag locations per image
N_CORES = 8
N_IMG = N // N_CORES    # images per core
JR = 64                 # person rows per core (2 images x 32, rows 30/31 dead)
CB = 64.0               # c; c^2 = 4096 exact

# feature flags (validated by HW probes; flip off to use safe fallbacks)
USE_NEWTON = True       # DVE bit-trick rsqrt instead of Sqrt activation table
USE_TTR = False         # fused tensor_tensor_reduce: custom-DVE op, crashes this runtime
USE_STT = True          # fused scalar_tensor_tensor
USE_HDUAL = True        # single strided dual-column write of h into Wa and Wb
USE_F32R = False        # verifier requires producer-side fp32r rounding

# TLC (constant-block) column layout, [66, TLC_W]
C_IDENT = 0      # 0:64 identity for the PE transpose
C_IND = 64       # 64:66 image one-hot; rows 64/65 = eye(2) pseudo
C_WA = 68        # 68:88  Wa: [-mean(16) | h | 1 | c*ind(2)]
C_WB = 100       # 100:120 Wb: [ mean(16) | 1 | h | -c*ind(2)]
C_TGT = 132      # 132:148 normalized scale target
C_RECIP = 148
C_NR = 149
C_RRVP = 150
C_H0 = 151
C_HV2 = 152
C_VRN = 153
C_CPUSH = 154
C_NVRN = 155
C_STAT = 156     # 156:159 stat cols: pull | push | scale; rows 64/65 pseudo
TLC_W = 160


def build_nc(n_blk=5):
    """n_blk gather blocks of 128 descriptors each."""
    nc = bacc.Bacc("TRN2", target_bir_lowering=False, debug=False)

    tags = nc.dram_tensor("tags", [N_IMG * L, S], F32, kind="ExternalInput")
    j2d = nc.dram_tensor("j2", [128, n_blk], I32, kind="ExternalInput")
    seld = nc.dram_tensor("sel", [128, n_blk * JR], F32, kind="ExternalInput")
    tlcd = nc.dram_tensor("tlc", [JR + 2, TLC_W], F32, kind="ExternalInput")
    out = nc.dram_tensor("out", [N_IMG, 3], F32, kind="ExternalOutput")

    with tile.TileContext(nc) as tc:
        with (
            tc.tile_pool(name="sb", bufs=1) as sb,
            tc.tile_pool(name="ps", bufs=1, space="PSUM") as ps,
        ):
            # ---- loads. j2 gates the gather stream -> first on Act queue;
            # sel on the idle sync queue; one hidden ACT table load (Exp
            # set, which also contains Abs) via the warmup activation ----
            j2 = sb.tile([128, n_blk], I32, tag="j2")
            nc.scalar.dma_start(j2[:], j2d.ap())
            tlc = sb.tile([JR + 2, TLC_W], F32, tag="tlc")
            nc.scalar.dma_start(tlc[:], tlcd.ap())
            sel = sb.tile([128, n_blk * JR], F32, tag="sel")
            nc.sync.dma_start(sel[:], seld.ap())

            warm = sb.tile([1, 2], F32, tag="warm")
            nc.vector.memset(warm[:, 0:1], 1.0)
            nc.scalar.activation(out=warm[:, 1:2], in_=warm[:, 0:1], func=AF.Exp)

            # ---- gather stream + per-block accumulation ----
            # Per block: mmU accumulates U = sel_c^T @ G_c straight off the
            # landed data (no DVE hop on the critical path); mmV accumulates
            # V = sel_c^T @ G_c^2 from the DVE square.
            F32M = mybir.dt.float32r if USE_F32R else F32
            GS = sb.tile([128, 2 * S * n_blk], F32, tag="GS")
            uvU = ps.tile([JR, S], F32, tag="uvU")
            uvV = ps.tile([JR, S], F32, tag="uvV")
            for c in range(n_blk):
                nc.gpsimd.indirect_dma_start(
                    out=GS[:, c * S : (c + 1) * S],
                    out_offset=None,
                    in_=tags.ap(),
                    in_offset=IndirectOffsetOnAxis(ap=j2[:, c : c + 1], axis=0),
                )
            sqo = n_blk * S
            for c in range(n_blk):
                g_c = GS[:, c * S : (c + 1) * S]
                sq_c = GS[:, sqo + c * S : sqo + (c + 1) * S]
                nc.vector.tensor_mul(out=sq_c, in0=g_c, in1=g_c)
                nc.tensor.matmul(
                    out=uvU[:],
                    lhsT=sel[:, c * JR : (c + 1) * JR].bitcast(F32M),
                    rhs=g_c.bitcast(F32M),
                    start=(c == 0),
                    stop=(c == n_blk - 1),
                )
                nc.tensor.matmul(
                    out=uvV[:],
                    lhsT=sel[:, c * JR : (c + 1) * JR].bitcast(F32M),
                    rhs=sq_c.bitcast(F32M),
                    start=(c == 0),
                    stop=(c == n_blk - 1),
                )

            # ---- per-person stats (one SBUF staging copy of U) ----
            tj = tlc[0:JR, :]
            U2 = sb.tile([JR, S], F32, tag="U2")
            nc.vector.tensor_copy(out=U2[:], in_=uvU[:])
            nc.vector.tensor_scalar_mul(
                out=tj[:, C_WB : C_WB + S], in0=U2[:],
                scalar1=tj[:, C_RECIP : C_RECIP + 1],
            )
            nc.vector.tensor_scalar_mul(
                out=tj[:, C_WA : C_WA + S], in0=U2[:],
                scalar1=tj[:, C_NR : C_NR + 1],
            )
            q = sb.tile([JR, 1], F32, tag="q")
            scrq = sb.tile([JR, S], F32, tag="scrq")
            nc.vector.tensor_mul(out=scrq[:], in0=U2[:], in1=U2[:])
            nc.vector.reduce_sum(out=q[:], in_=scrq[:], axis=mybir.AxisListType.X)
            absU = sb.tile([JR, S], F32, tag="absU")
            nc.scalar.activation(out=absU[:], in_=uvU[:], func=AF.Abs)
            sg = sb.tile([JR, 1], F32, tag="sg")
            nc.vector.reduce_sum(out=sg[:], in_=uvV[:], axis=mybir.AxisListType.X)
            scr16 = sb.tile([JR, S], F32, tag="scr16")

            if USE_HDUAL:
                hview = tj[:, C_WA + S : C_WA + S + 2 * 33].rearrange(
                    "p (a b) -> p a b", a=2
                )[:, :, 0:1]
                qb = q[:].rearrange("p (a b) -> p a b", a=1).to_broadcast(
                    [JR, 2, 1]
                )
                nc.vector.tensor_scalar(
                    out=hview, in0=qb,
                    scalar1=tj[:, C_H0 : C_H0 + 1], scalar2=tj[:, C_HV2 : C_HV2 + 1],
                    op0=ALU.mult, op1=ALU.add,
                )
            else:
                nc.vector.tensor_scalar(
                    out=tj[:, C_WA + S : C_WA + S + 1], in0=q[:],
                    scalar1=tj[:, C_H0 : C_H0 + 1], scalar2=tj[:, C_HV2 : C_HV2 + 1],
                    op0=ALU.mult, op1=ALU.add,
                )
                nc.scalar.copy(
                    out=tj[:, C_WB + S + 1 : C_WB + S + 2],
                    in_=tj[:, C_WA + S : C_WA + S + 1],
                )

            # ---- push: one transpose, two aligned copies, Gram, Exp ----
            tp = ps.tile([64, JR], F32, tag="tp")
            nc.tensor.transpose(
                out=tp[:], in_=tj[:, C_WA : C_WA + 64], identity=tj[:, 0:JR]
            )
            X = sb.tile([20, JR], F32, tag="X")
            nc.vector.tensor_copy(out=X[:], in_=tp[0:20, :])
            Y = sb.tile([20, JR], F32, tag="Y")
            nc.scalar.copy(out=Y[:], in_=tp[32:52, :])
            F32M2 = mybir.dt.float32r if USE_F32R else F32
            dh = ps.tile([JR, JR], F32, tag="dh")
            nc.tensor.matmul(
                out=dh[:], lhsT=X[:].bitcast(F32M2), rhs=Y[:].bitcast(F32M2),
                start=True, stop=True,
            )

            # ---- scale branch: rsqrt(q) ----
            rq = sb.tile([JR, 1], F32, tag="rq")
            mq = sb.tile([JR, 1], F32, tag="mq")
            nc.vector.tensor_scalar_max(out=mq[:], in0=q[:], scalar1=1e-30)
            if USE_NEWTON:
                ti = sb.tile([JR, 1], I32, tag="ti")
                nc.vector.tensor_single_scalar(
                    out=ti[:], in_=mq[:].bitcast(I32), scalar=1,
                    op=ALU.logical_shift_right,
                )
                yi = sb.tile([JR, 1], I32, tag="yi")
                nc.vector.tensor_scalar(
                    out=yi[:], in0=ti[:], scalar1=-1, scalar2=0x5F3759DF,
                    op0=ALU.mult, op1=ALU.add,
                )
                y0 = yi[:].bitcast(F32)
                y2 = sb.tile([JR, 1], F32, tag="y2")
                e = sb.tile([JR, 1], F32, tag="e")
                f = sb.tile([JR, 1], F32, tag="f")
                nc.vector.tensor_mul(out=y2[:], in0=y0, in1=y0)
                nc.vector.tensor_mul(out=e[:], in0=mq[:], in1=y2[:])
                nc.vector.tensor_scalar(
                    out=f[:], in0=e[:], scalar1=-0.5, scalar2=1.5,
                    op0=ALU.mult, op1=ALU.add,
                )
                # one Newton step: rel err ~2e-3 (seed ~3.4%), inside budget
                nc.vector.tensor_mul(out=rq[:], in0=y0, in1=f[:])
            else:
                sq_ = sb.tile([JR, 1], F32, tag="sq_")
                nc.scalar.sqrt(out=sq_[:], in_=mq[:])
                nc.vector.reciprocal(out=rq[:], in_=sq_[:])

            A = sb.tile([JR, 1], F32, tag="A")
            if USE_TTR:
                nc.vector.tensor_tensor_reduce(
                    out=scr16[:], in0=absU[:], in1=tj[:, C_TGT : C_TGT + S],
                    scale=1.0, scalar=0.0, op0=ALU.mult, op1=ALU.add,
                    accum_out=A[:],
                )
            else:
                nc.vector.tensor_mul(
                    out=scr16[:], in0=absU[:], in1=tj[:, C_TGT : C_TGT + S]
                )
                nc.vector.reduce_sum(
                    out=A[:], in_=scr16[:], axis=mybir.AxisListType.X
                )
            d12 = sb.tile([JR, 1], F32, tag="d12")
            nc.vector.tensor_mul(out=d12[:], in0=A[:], in1=rq[:])
            # scale stat = vrn - d12*vrn
            if USE_STT:
                nc.vector.scalar_tensor_tensor(
                    out=tj[:, C_STAT + 2 : C_STAT + 3], in0=d12[:],
                    scalar=tj[:, C_NVRN : C_NVRN + 1],
                    in1=tj[:, C_VRN : C_VRN + 1],
                    op0=ALU.mult, op1=ALU.add,
                )
            else:
                t2 = sb.tile([JR, 1], F32, tag="t2")
                nc.vector.tensor_scalar(
                    out=t2[:], in0=d12[:], scalar1=tj[:, C_NVRN : C_NVRN + 1],
                    scalar2=None, op0=ALU.mult,
                )
                nc.vector.tensor_scalar(
                    out=tj[:, C_STAT + 2 : C_STAT + 3], in0=t2[:],
                    scalar1=tj[:, C_VRN : C_VRN + 1], scalar2=None, op0=ALU.add,
                )
            # pull stat = (q*nr + sg) * rrvp
            p1 = sb.tile([JR, 1], F32, tag="p1")
            if USE_STT:
                nc.vector.scalar_tensor_tensor(
                    out=p1[:], in0=q[:], scalar=tj[:, C_NR : C_NR + 1],
                    in1=sg[:], op0=ALU.mult, op1=ALU.add,
                )
            else:
                t3 = sb.tile([JR, 1], F32, tag="t3")
                nc.vector.tensor_scalar(
                    out=t3[:], in0=q[:], scalar1=tj[:, C_NR : C_NR + 1],
                    scalar2=None, op0=ALU.mult,
                )
                nc.vector.tensor_add(out=p1[:], in0=t3[:], in1=sg[:])
            nc.vector.tensor_scalar_mul(
                out=tj[:, C_STAT : C_STAT + 1], in0=p1[:],
                scalar1=tj[:, C_RRVP : C_RRVP + 1],
            )

            # push stat: row-sums of exp(-2*Gram + ln(cpush)) accumulate
            # straight into the stat column (bias folds the cpush scale)
            eo = sb.tile([JR, JR], F32, tag="eo")
            nc.scalar.activation(
                out=eo[:], in_=dh[:], func=AF.Exp, scale=-2.0,
                bias=tj[:, C_CPUSH : C_CPUSH + 1],
                accum_out=tj[:, C_STAT + 1 : C_STAT + 2],
            )

            # ---- per-image reduction (pseudo-rows fold the diagonal fix) ----
            fsp = ps.tile([N_IMG, 3], F32, tag="fsp")
            nc.tensor.matmul(
                out=fsp[:], lhsT=tlc[:, C_IND : C_IND + 2],
                rhs=tlc[:, C_STAT : C_STAT + 3], start=True, stop=True,
            )
            ob = sb.tile([N_IMG, 3], F32, tag="ob")
            nc.vector.tensor_copy(out=ob[:], in_=fsp[:])
            nc.sync.dma_start(out.ap(), ob[:])

    nc.compile()
    return nc


def _prep_core(tags_c, joints_c, box_c, sd, n_blk):
    """Host-side shard prep: compacted visible-joint gather list, per-block
    selectors, and the constant block. Returns None if the visible count
    exceeds this build's capacity (caller rebuilds with more blocks)."""
    cap = 128 * n_blk
    tags2 = np.ascontiguousarray(
        np.asarray(tags_c, dtype=np.float32).reshape(N_IMG * L, S)
    )
    jl = np.asarray(joints_c[..., 0], dtype=np.int64)      # [2, 30, 17]
    vis = np.asarray(joints_c[..., 1]) > 0
    loc = (jl + (np.arange(N_IMG) * L)[:, None, None]).astype(np.int64)

    img_r, m_r, k_r = np.nonzero(vis)          # visible (img, person, joint)
    V = img_r.shape[0]
    if V > cap:
        return None
    jrow = img_r * 32 + m_r                    # person row 0..63
    locv = loc[img_r, m_r, k_r].astype(np.int32)

    j2 = np.zeros((128, n_blk), np.int32)
    selm = np.zeros((128, n_blk * JR), np.float32)
    fi = np.arange(V)
    p_i, c_i = fi % 128, fi // 128
    j2[p_i, c_i] = locv
    selm[p_i, c_i * JR + jrow] = 1.0

    visf = vis.reshape(N_IMG * M, K).astype(np.float32)
    cnt_pm = visf.sum(1).reshape(N_IMG, M)     # [2, 30]
    cnt = np.zeros((N_IMG, 32), np.float32)
    cnt[:, 0:M] = cnt_pm
    cnt = cnt.reshape(JR)
    recip = (1.0 / np.maximum(cnt, 1.0)).astype(np.float32)
    valid = (cnt > 0).astype(np.float32)
    imgr = np.arange(JR) // 32
    nv = np.array([valid[imgr == i].sum() for i in range(N_IMG)], np.float32)
    rn = (1.0 / np.maximum(nv, 1.0)).astype(np.float32)
    rp = (1.0 / np.maximum(nv * (nv - 1.0), 1.0)).astype(np.float32)
    ge2 = (nv >= 2.0).astype(np.float32)
    cpush = 0.5 * rp * ge2

    box = np.zeros((N_IMG, 32), np.float32)
    box[:, 0:M] = np.asarray(box_c, dtype=np.float32).reshape(N_IMG, M)
    box = box.reshape(JR)
    sd = np.asarray(sd, dtype=np.float32).reshape(S)
    gap = np.abs(box[:, None] - sd[None, :]).astype(np.float32)
    r = (np.float32(1.0) / (gap + np.float32(1e-10))).astype(np.float32)
    nrm = np.sqrt((r * r).sum(1, dtype=np.float32))
    tgt = r / np.maximum(nrm, np.float32(1e-12))[:, None]

    tlc = np.zeros((JR + 2, TLC_W), np.float32)
    pj = np.arange(JR)
    tlc[0:JR, 0:JR] = np.eye(JR, dtype=np.float32)
    tlc[pj, C_IND + imgr] = 1.0
    tlc[JR, C_IND] = 1.0
    tlc[JR + 1, C_IND + 1] = 1.0
    tlc[0:JR, C_WA + S + 1] = 1.0
    tlc[pj, C_WA + S + 2 + imgr] = CB
    tlc[0:JR, C_WB + S] = 1.0
    tlc[pj, C_WB + S + 2 + imgr] = -CB
    tlc[0:JR, C_TGT : C_TGT + S] = tgt
    tlc[0:JR, C_RECIP] = recip
    tlc[0:JR, C_NR] = -recip
    tlc[0:JR, C_RRVP] = (recip / S) * valid * rn[imgr]
    tlc[0:JR, C_H0] = 0.5 * recip * recip
    tlc[0:JR, C_HV2] = 4096.0 * (1.0 - valid) + 2048.0
    tlc[0:JR, C_VRN] = valid * rn[imgr]
    lncp = np.where(cpush > 0, np.log(np.maximum(cpush, 1e-38)), -1e30).astype(
        np.float32
    )
    tlc[0:JR, C_CPUSH] = lncp[imgr]
    tlc[0:JR, C_NVRN] = -tlc[0:JR, C_VRN]
    tlc[JR, C_STAT + 1] = -cpush[0] * nv[0]
    tlc[JR + 1, C_STAT + 1] = -cpush[1] * nv[1]
    return {"tags": tags2, "j2": j2, "sel": selm, "tlc": tlc}


_NC_CACHE = {}


def _get_nc(n_blk):
    if n_blk not in _NC_CACHE:
        _NC_CACHE[n_blk] = build_nc(n_blk)
    return _NC_CACHE[n_blk]


def kernel(tags, joints, box_scales, scale_dist, _trace=False):
    """Full-input entry point; shards across 8 NeuronCores and gathers."""
    tags = np.asarray(tags)
    joints = np.asarray(joints)
    box_scales = np.asarray(box_scales)
    scale_dist = np.asarray(scale_dist)

    for n_blk in (5, 9):  # 9-block fallback only if >640 joints are visible
        in_maps = [
            _prep_core(
                tags[N_IMG * c : N_IMG * (c + 1)],
                joints[N_IMG * c : N_IMG * (c + 1)],
                box_scales[N_IMG * c : N_IMG * (c + 1)],
                scale_dist,
                n_blk,
            )
            for c in range(N_CORES)
        ]
        if all(m is not None for m in in_maps):
            break

    res = run_bass_kernel_spmd(
        _get_nc(n_blk), in_maps, core_ids=list(range(N_CORES)), trace=_trace
    )
    parts = np.concatenate(
        [res.results[c]["out"] for c in range(N_CORES)], axis=0
    )  # [N, 3]
    final = parts.mean(axis=0).astype(np.float32)
    if _trace:
        return final, res
    return final


# revision 11
# speedup vs baseline: 1.0664x; 1.0050x over previous
"""AssociativeEmbeddingLoss on 8 TRN2 NeuronCores (Bass/Tile kernel) — v3.

Entry point: kernel(**inputs) -> np.ndarray (3,) = (pull, push, scale),
matching the reference. Data-parallel on batch dim N=16 -> 2 images per
core; per-image partials are averaged on the host.

Design (vs the 39.4us 9-call baseline):
  - Only VISIBLE joints are gathered: invisible ones are multiplied by
    zero downstream anyway, so the host compacts the ~1020 (person,
    joint) pairs to the ~510 visible ones. 640 descriptor capacity
    (8 sigma above the Binomial(1020,1/2) mean) -> FIVE indirect-DMA
    calls instead of nine; a >640 overflow (never for random inputs)
    falls back to a lazily-built 9-call variant.
  - The visibility mask is folded into per-block 0/1 selector matrices:
    as each 128-descriptor gather block lands, one PE matmul
    accumulates both U = sum(vis*g) and V = sum(vis*g^2) into PSUM
    ([64,32], rhs = [G_c | G_c^2] via a strided view), entirely hidden
    under the remaining gather stream. No 128->60 half-merge needed.
  - Everything derivable from visibility counts / box_scales alone
    (reciprocals, valid masks, normalized scale targets tgt, per-image
    1/n factors) is precomputed on the host into one constant-block
    DMA. The device only computes what needs gathered tag values.
  - All activations used (Exp/Abs) live in one activation-table set, so
    one hidden table load replaces the baseline's five (two of which
    sat on the critical path).
  - rsqrt(||U||^2) for the scale loss is a DVE Newton iteration seeded
    by the int32 bit trick - no Sqrt table set, no Act round-trip.
  - The push-loss pair mask folds into the Gram matmul via +-64.0
    image-indicator feature columns (64^2=4096 exactly cancels the 2048
    h-constants in fp32); the diagonal exp(0)=1 surplus per valid
    person is cancelled by host-constant pseudo-rows in the final
    per-image reduction matmul.
"""

import numpy as np

import concourse.bacc as bacc
import concourse.mybir as mybir
import concourse.tile as tile
from concourse.bass import IndirectOffsetOnAxis
from concourse.bass_utils import run_bass_kernel_spmd

F32 = mybir.dt.float32
I32 = mybir.dt.int32
AF = mybir.ActivationFunctionType
ALU = mybir.AluOpType

S = 16      # scale-embedding dim
K = 17      # joints
M = 30      # persons per image
N = 16      # batch
L = 69632   # flattened tag locations per image
N_CORES = 8
N_IMG = N // N_CORES    # images per core
JR = 64                 # person rows per core (2 images x 32, rows 30/31 dead)
CB = 64.0               # c; c^2 = 4096 exact

# feature flags (validated by HW probes; flip off to use safe fallbacks)
USE_NEWTON = True       # DVE bit-trick rsqrt instead of Sqrt activation table
USE_TTR = False         # fused tensor_tensor_reduce: custom-DVE op, crashes this runtime
USE_STT = True          # fused scalar_tensor_tensor
USE_HDUAL = True        # single strided dual-column write of h into Wa and Wb
USE_F32R = False        # verifier requires producer-side fp32r rounding

# TLC (constant-block) column layout, [66, TLC_W]
C_IDENT = 0      # 0:64 identity for the PE transpose
C_IND = 64       # 64:66 image one-hot; rows 64/65 = eye(2) pseudo
C_WA = 68        # 68:88  Wa: [-mean(16) | h | 1 | c*ind(2)]
C_WB = 100       # 100:120 Wb: [ mean(16) | 1 | h | -c*ind(2)]
C_TGT = 132      # 132:148 normalized scale target
C_RECIP = 148
C_NR = 149
C_RRVP = 150
C_H0 = 151
C_HV2 = 152
C_VRN = 153
C_CPUSH = 154
C_NVRN = 155
C_STAT = 156     # 156:159 stat cols: pull | push | scale; rows 64/65 pseudo
TLC_W = 160


def build_nc(n_blk=5):
    """n_blk gather blocks of 128 descriptors each."""
    nc = bacc.Bacc("TRN2", target_bir_lowering=False, debug=False)

    tags = nc.dram_tensor("tags", [N_IMG * L, S], F32, kind="ExternalInput")
    j2d = nc.dram_tensor("j2", [128, n_blk], I32, kind="ExternalInput")
    seld = nc.dram_tensor("sel", [128, n_blk * JR], F32, kind="ExternalInput")
    tlcd = nc.dram_tensor("tlc", [JR + 2, TLC_W], F32, kind="ExternalInput")
    out = nc.dram_tensor("out", [N_IMG, 3], F32, kind="ExternalOutput")

    with tile.TileContext(nc) as tc:
        with (
            tc.tile_pool(name="sb", bufs=1) as sb,
            tc.tile_pool(name="ps", bufs=1, space="PSUM") as ps,
        ):
            # ---- loads. j2 gates the gather stream -> first on Act queue;
            # sel on the idle sync queue; one hidden ACT table load (Exp
            # set, which also contains Abs) via the warmup activation ----
            j2 = sb.tile([128, n_blk], I32, tag="j2")
            nc.scalar.dma_start(j2[:], j2d.ap())
            tlc = sb.tile([JR + 2, TLC_W], F32, tag="tlc")
            nc.scalar.dma_start(tlc[:], tlcd.ap())
            sel = sb.tile([128, n_blk * JR], F32, tag="sel")
            nc.sync.dma_start(sel[:], seld.ap())

            warm = sb.tile([1, 2], F32, tag="warm")
            nc.vector.memset(warm[:, 0:1], 1.0)
            nc.scalar.activation(out=warm[:, 1:2], in_=warm[:, 0:1], func=AF.Exp)

            # ---- gather stream + per-block accumulation ----
            # Per block: mmU accumulates U = sel_c^T @ G_c straight off the
            # landed data (no DVE hop on the critical path); mmV accumulates
            # V = sel_c^T @ G_c^2 from the DVE square.
            F32M = mybir.dt.float32r if USE_F32R else F32
            GS = sb.tile([128, 2 * S * n_blk], F32, tag="GS")
            uvU = ps.tile([JR, S], F32, tag="uvU")
            uvV = ps.tile([JR, S], F32, tag="uvV")
            for c in range(n_blk):
                nc.gpsimd.indirect_dma_start(
                    out=GS[:, c * S : (c + 1) * S],
                    out_offset=None,
                    in_=tags.ap(),
                    in_offset=IndirectOffsetOnAxis(ap=j2[:, c : c + 1], axis=0),
                )
            sqo = n_blk * S
            for c in range(n_blk):
                g_c = GS[:, c * S : (c + 1) * S]
                sq_c = GS[:, sqo + c * S : sqo + (c + 1) * S]
                nc.vector.tensor_mul(out=sq_c, in0=g_c, in1=g_c)
                nc.tensor.matmul(
                    out=uvU[:],
                    lhsT=sel[:, c * JR : (c + 1) * JR].bitcast(F32M),
                    rhs=g_c.bitcast(F32M),
                    start=(c == 0),
                    stop=(c == n_blk - 1),
                )
                nc.tensor.matmul(
                    out=uvV[:],
                    lhsT=sel[:, c * JR : (c + 1) * JR].bitcast(F32M),
                    rhs=sq_c.bitcast(F32M),
                    start=(c == 0),
                    stop=(c == n_blk - 1),
                )

            # ---- per-person stats; q via an Act-side Square so the DVE
            # pre-transpose chain is just Wb -> qreduce -> h, with the Wa
            # columns produced in parallel on Act as -Wb ----
            tj = tlc[0:JR, :]
            nc.vector.tensor_scalar_mul(
                out=tj[:, C_WB : C_WB + S], in0=uvU[:],
                scalar1=tj[:, C_RECIP : C_RECIP + 1],
            )
            q = sb.tile([JR, 1], F32, tag="q")
            scrq = sb.tile([JR, S], F32, tag="scrq")
            nc.scalar.activation(out=scrq[:], in_=uvU[:], func=AF.Square)
            nc.vector.reduce_sum(out=q[:], in_=scrq[:], axis=mybir.AxisListType.X)
            nc.scalar.mul(
                out=tj[:, C_WA : C_WA + S], in_=tj[:, C_WB : C_WB + S], mul=-1.0
            )
            absU = sb.tile([JR, S], F32, tag="absU")
            nc.scalar.activation(out=absU[:], in_=uvU[:], func=AF.Abs)
            sg = sb.tile([JR, 1], F32, tag="sg")
            nc.vector.reduce_sum(out=sg[:], in_=uvV[:], axis=mybir.AxisListType.X)
            scr16 = sb.tile([JR, S], F32, tag="scr16")

            if USE_HDUAL:
                hview = tj[:, C_WA + S : C_WA + S + 2 * 33].rearrange(
                    "p (a b) -> p a b", a=2
                )[:, :, 0:1]
                qb = q[:].rearrange("p (a b) -> p a b", a=1).to_broadcast(
                    [JR, 2, 1]
                )
                nc.vector.tensor_scalar(
                    out=hview, in0=qb,
                    scalar1=tj[:, C_H0 : C_H0 + 1], scalar2=tj[:, C_HV2 : C_HV2 + 1],
                    op0=ALU.mult, op1=ALU.add,
                )
            else:
                nc.vector.tensor_scalar(
                    out=tj[:, C_WA + S : C_WA + S + 1], in0=q[:],
                    scalar1=tj[:, C_H0 : C_H0 + 1], scalar2=tj[:, C_HV2 : C_HV2 + 1],
                    op0=ALU.mult, op1=ALU.add,
                )
                nc.scalar.copy(
                    out=tj[:, C_WB + S + 1 : C_WB + S + 2],
                    in_=tj[:, C_WA + S : C_WA + S + 1],
                )

            # ---- push: one transpose, two aligned copies, Gram, Exp ----
            tp = ps.tile([64, JR], F32, tag="tp")
            nc.tensor.transpose(
                out=tp[:], in_=tj[:, C_WA : C_WA + 64], identity=tj[:, 0:JR]
            )
            X = sb.tile([20, JR], F32, tag="X")
            nc.vector.tensor_copy(out=X[:], in_=tp[0:20, :])
            Y = sb.tile([20, JR], F32, tag="Y")
            nc.scalar.copy(out=Y[:], in_=tp[32:52, :])
            F32M2 = mybir.dt.float32r if USE_F32R else F32
            dh = ps.tile([JR, JR], F32, tag="dh")
            nc.tensor.matmul(
                out=dh[:], lhsT=X[:].bitcast(F32M2), rhs=Y[:].bitcast(F32M2),
                start=True, stop=True,
            )

            # ---- scale branch: rsqrt(q) ----
            rq = sb.tile([JR, 1], F32, tag="rq")
            mq = sb.tile([JR, 1], F32, tag="mq")
            nc.vector.tensor_scalar_max(out=mq[:], in0=q[:], scalar1=1e-30)
            if USE_NEWTON:
                ti = sb.tile([JR, 1], I32, tag="ti")
                nc.vector.tensor_single_scalar(
                    out=ti[:], in_=mq[:].bitcast(I32), scalar=1,
                    op=ALU.logical_shift_right,
                )
                yi = sb.tile([JR, 1], I32, tag="yi")
                nc.vector.tensor_scalar(
                    out=yi[:], in0=ti[:], scalar1=-1, scalar2=0x5F3759DF,
                    op0=ALU.mult, op1=ALU.add,
                )
                y0 = yi[:].bitcast(F32)
                y2 = sb.tile([JR, 1], F32, tag="y2")
                e = sb.tile([JR, 1], F32, tag="e")
                f = sb.tile([JR, 1], F32, tag="f")
                nc.vector.tensor_mul(out=y2[:], in0=y0, in1=y0)
                nc.vector.tensor_mul(out=e[:], in0=mq[:], in1=y2[:])
                nc.vector.tensor_scalar(
                    out=f[:], in0=e[:], scalar1=-0.5, scalar2=1.5,
                    op0=ALU.mult, op1=ALU.add,
                )
                # one Newton step: rel err ~2e-3 (seed ~3.4%), inside budget
                nc.vector.tensor_mul(out=rq[:], in0=y0, in1=f[:])
            else:
                sq_ = sb.tile([JR, 1], F32, tag="sq_")
                nc.scalar.sqrt(out=sq_[:], in_=mq[:])
                nc.vector.reciprocal(out=rq[:], in_=sq_[:])

            A = sb.tile([JR, 1], F32, tag="A")
            if USE_TTR:
                nc.vector.tensor_tensor_reduce(
                    out=scr16[:], in0=absU[:], in1=tj[:, C_TGT : C_TGT + S],
                    scale=1.0, scalar=0.0, op0=ALU.mult, op1=ALU.add,
                    accum_out=A[:],
                )
            else:
                nc.vector.tensor_mul(
                    out=scr16[:], in0=absU[:], in1=tj[:, C_TGT : C_TGT + S]
                )
                nc.vector.reduce_sum(
                    out=A[:], in_=scr16[:], axis=mybir.AxisListType.X
                )
            d12 = sb.tile([JR, 1], F32, tag="d12")
            nc.vector.tensor_mul(out=d12[:], in0=A[:], in1=rq[:])
            # scale stat = vrn - d12*vrn
            if USE_STT:
                nc.vector.scalar_tensor_tensor(
                    out=tj[:, C_STAT + 2 : C_STAT + 3], in0=d12[:],
                    scalar=tj[:, C_NVRN : C_NVRN + 1],
                    in1=tj[:, C_VRN : C_VRN + 1],
                    op0=ALU.mult, op1=ALU.add,
                )
            else:
                t2 = sb.tile([JR, 1], F32, tag="t2")
                nc.vector.tensor_scalar(
                    out=t2[:], in0=d12[:], scalar1=tj[:, C_NVRN : C_NVRN + 1],
                    scalar2=None, op0=ALU.mult,
                )
                nc.vector.tensor_scalar(
                    out=tj[:, C_STAT + 2 : C_STAT + 3], in0=t2[:],
                    scalar1=tj[:, C_VRN : C_VRN + 1], scalar2=None, op0=ALU.add,
                )
            # pull stat = (q*nr + sg) * rrvp
            p1 = sb.tile([JR, 1], F32, tag="p1")
            if USE_STT:
                nc.vector.scalar_tensor_tensor(
                    out=p1[:], in0=q[:], scalar=tj[:, C_NR : C_NR + 1],
                    in1=sg[:], op0=ALU.mult, op1=ALU.add,
                )
            else:
                t3 = sb.tile([JR, 1], F32, tag="t3")
                nc.vector.tensor_scalar(
                    out=t3[:], in0=q[:], scalar1=tj[:, C_NR : C_NR + 1],
                    scalar2=None, op0=ALU.mult,
                )
                nc.vector.tensor_add(out=p1[:], in0=t3[:], in1=sg[:])
            nc.vector.tensor_scalar_mul(
                out=tj[:, C_STAT : C_STAT + 1], in0=p1[:],
                scalar1=tj[:, C_RRVP : C_RRVP + 1],
            )

            # push stat: row-sums of exp(-2*Gram + ln(cpush)) accumulate
            # straight into the stat column (bias folds the cpush scale)
            eo = sb.tile([JR, JR], F32, tag="eo")
            nc.scalar.activation(
                out=eo[:], in_=dh[:], func=AF.Exp, scale=-2.0,
                bias=tj[:, C_CPUSH : C_CPUSH + 1],
                accum_out=tj[:, C_STAT + 1 : C_STAT + 2],
            )

            # ---- per-image reduction (pseudo-rows fold the diagonal fix) ----
            fsp = ps.tile([N_IMG, 3], F32, tag="fsp")
            nc.tensor.matmul(
                out=fsp[:], lhsT=tlc[:, C_IND : C_IND + 2],
                rhs=tlc[:, C_STAT : C_STAT + 3], start=True, stop=True,
            )
            ob = sb.tile([N_IMG, 3], F32, tag="ob")
            nc.vector.tensor_copy(out=ob[:], in_=fsp[:])
            nc.sync.dma_start(out.ap(), ob[:])

    nc.compile()
    return nc


def _prep_core(tags_c, joints_c, box_c, sd, n_blk):
    """Host-side shard prep: compacted visible-joint gather list, per-block
    selectors, and the constant block. Returns None if the visible count
    exceeds this build's capacity (caller rebuilds with more blocks)."""
    cap = 128 * n_blk
    tags2 = np.ascontiguousarray(
        np.asarray(tags_c, dtype=np.float32).reshape(N_IMG * L, S)
    )
    jl = np.asarray(joints_c[..., 0], dtype=np.int64)      # [2, 30, 17]
    vis = np.asarray(joints_c[..., 1]) > 0
    loc = (jl + (np.arange(N_IMG) * L)[:, None, None]).astype(np.int64)

    img_r, m_r, k_r = np.nonzero(vis)          # visible (img, person, joint)
    V = img_r.shape[0]
    if V > cap:
        return None
    jrow = img_r * 32 + m_r                    # person row 0..63
    locv = loc[img_r, m_r, k_r].astype(np.int32)

    j2 = np.zeros((128, n_blk), np.int32)
    selm = np.zeros((128, n_blk * JR), np.float32)
    fi = np.arange(V)
    p_i, c_i = fi % 128, fi // 128
    j2[p_i, c_i] = locv
    selm[p_i, c_i * JR + jrow] = 1.0

    visf = vis.reshape(N_IMG * M, K).astype(np.float32)
    cnt_pm = visf.sum(1).reshape(N_IMG, M)     # [2, 30]
    cnt = np.zeros((N_IMG, 32), np.float32)
    cnt[:, 0:M] = cnt_pm
    cnt = cnt.reshape(JR)
    recip = (1.0 / np.maximum(cnt, 1.0)).astype(np.float32)
    valid = (cnt > 0).astype(np.float32)
    imgr = np.arange(JR) // 32
    nv = np.array([valid[imgr == i].sum() for i in range(N_IMG)], np.float32)
    rn = (1.0 / np.maximum(nv, 1.0)).astype(np.float32)
    rp = (1.0 / np.maximum(nv * (nv - 1.0), 1.0)).astype(np.float32)
    ge2 = (nv >= 2.0).astype(np.float32)
    cpush = 0.5 * rp * ge2

    box = np.zeros((N_IMG, 32), np.float32)
    box[:, 0:M] = np.asarray(box_c, dtype=np.float32).reshape(N_IMG, M)
    box = box.reshape(JR)
    sd = np.asarray(sd, dtype=np.float32).reshape(S)
    gap = np.abs(box[:, None] - sd[None, :]).astype(np.float32)
    r = (np.float32(1.0) / (gap + np.float32(1e-10))).astype(np.float32)
    nrm = np.sqrt((r * r).sum(1, dtype=np.float32))
    tgt = r / np.maximum(nrm, np.float32(1e-12))[:, None]

    tlc = np.zeros((JR + 2, TLC_W), np.float32)
    pj = np.arange(JR)
    tlc[0:JR, 0:JR] = np.eye(JR, dtype=np.float32)
    tlc[pj, C_IND + imgr] = 1.0
    tlc[JR, C_IND] = 1.0
    tlc[JR + 1, C_IND + 1] = 1.0
    tlc[0:JR, C_WA + S + 1] = 1.0
    tlc[pj, C_WA + S + 2 + imgr] = CB
    tlc[0:JR, C_WB + S] = 1.0
    tlc[pj, C_WB + S + 2 + imgr] = -CB
    tlc[0:JR, C_TGT : C_TGT + S] = tgt
    tlc[0:JR, C_RECIP] = recip
    tlc[0:JR, C_NR] = -recip
    tlc[0:JR, C_RRVP] = (recip / S) * valid * rn[imgr]
    tlc[0:JR, C_H0] = 0.5 * recip * recip
    tlc[0:JR, C_HV2] = 4096.0 * (1.0 - valid) + 2048.0
    tlc[0:JR, C_VRN] = valid * rn[imgr]
    lncp = np.where(cpush > 0, np.log(np.maximum(cpush, 1e-38)), -1e30).astype(
        np.float32
    )
    tlc[0:JR, C_CPUSH] = lncp[imgr]
    tlc[0:JR, C_NVRN] = -tlc[0:JR, C_VRN]
    tlc[JR, C_STAT + 1] = -cpush[0] * nv[0]
    tlc[JR + 1, C_STAT + 1] = -cpush[1] * nv[1]
    return {"tags": tags2, "j2": j2, "sel": selm, "tlc": tlc}


_NC_CACHE = {}


def _get_nc(n_blk):
    if n_blk not in _NC_CACHE:
        _NC_CACHE[n_blk] = build_nc(n_blk)
    return _NC_CACHE[n_blk]


def kernel(tags, joints, box_scales, scale_dist, _trace=False):
    """Full-input entry point; shards across 8 NeuronCores and gathers."""
    tags = np.asarray(tags)
    joints = np.asarray(joints)
    box_scales = np.asarray(box_scales)
    scale_dist = np.asarray(scale_dist)

    for n_blk in (5, 9):  # 9-block fallback only if >640 joints are visible
        in_maps = [
            _prep_core(
                tags[N_IMG * c : N_IMG * (c + 1)],
                joints[N_IMG * c : N_IMG * (c + 1)],
                box_scales[N_IMG * c : N_IMG * (c + 1)],
                scale_dist,
                n_blk,
            )
            for c in range(N_CORES)
        ]
        if all(m is not None for m in in_maps):
            break

    res = run_bass_kernel_spmd(
        _get_nc(n_blk), in_maps, core_ids=list(range(N_CORES)), trace=_trace
    )
    parts = np.concatenate(
        [res.results[c]["out"] for c in range(N_CORES)], axis=0
    )  # [N, 3]
    final = parts.mean(axis=0).astype(np.float32)
    if _trace:
        return final, res
    return final
